# revision 1
# baseline (speedup 1.0000x reference)
"""Multi-head attention (nn_GroupQueryAttention_163208757512) on 8 TRN2 cores.

Problem: B=2, S=2048, D=1024, H=16 heads, DK=64. f32.
    q = Q @ Wq.T + bq  (per head)   k, v likewise
    out = softmax(q k^T / 8) v  -> concat heads -> @ Wo.T + bo

Sharding: core c handles batch b=c//4 and head group g=c%4 (4 heads,
feature slice hs = 256*g : 256*g+256). Data parallel on B, tensor
parallel on heads; the output projection yields per-core partials that
the host sums (replaces the all-reduce).

Device-side design (all PE matmuls in float32r: 1 cycle/row, ~1e-4 rounding):
  - host supplies X^T = {Q,K,V}[b].T so projections write q^T,k^T in
    [dh, s] layout directly; v is produced in natural [s, dh] layout.
  - scores are computed pre-transposed: S^T[sk, sq] = K_h Q_h^T, with
    two heads row-packed into the 128x128 PE array (K=64 each).
  - exp runs on ScalarE straight out of 2-bank PSUM with scale=1/8.
  - PV uses the stationary [v_h | 1] trick (M=65): column 64 accumulates
    the softmax denominators for free.
  - 1/denom is broadcast across partitions with a K=1 rank-1 matmul
    (ones^T x recip) and applied on VectorE during PSUM evacuation,
    which also adds bv. Result x_norm^T is the out-proj lhsT.
  - bo enters via a rank-1 ones x bo product added during output
    evacuation (only on the g==0 core of each batch).

Constraint discovered on this toolchain: walrus allows ONE sync-wait per
instruction, so a post-pass (split_waits) chains excess waits onto NoOps.
Accumulation groups must keep one lhsT base partition (HW fault otherwise).
"""

import os
import numpy as np
from contextlib import ExitStack

import concourse.bass as bass
import concourse.mybir as mybir
import concourse.tile as tile
from concourse.bass import ds, ts
from concourse.bass_utils import run_bass_kernel_spmd

F32 = mybir.dt.float32
F32R = mybir.dt.float32r
AF = mybir.ActivationFunctionType
ALU = mybir.AluOpType

B, S, D, H = 2, 2048, 1024, 16
DK = D // H            # 64
NCORES = 8
GROUPS = 4             # head groups per batch
DH = D // GROUPS       # 256 feature cols per core
P = 128
KT = D // P            # 8 contraction tiles for projections
ST = S // P            # 16 s-tiles
CH = 4                 # s-chunks
CW = S // CH           # 512


# ---------------------------------------------------------------- wait fix
_wf_counter = [0]


def _split_waits(nc, cap=1):
    """walrus in this container accepts at most one sync-wait command per
    instruction; chain the rest onto same-engine NoOps placed just before."""
    for fn in nc.m.functions:
        for bb in fn.blocks:
            out, changed = [], False
            for inst in bb.instructions:
                si = inst.sync_info
                waits = list(si.on_wait) if (si is not None and si.on_wait) else []
                if len(waits) > cap:
                    changed = True
                    keep = waits[-cap:]
                    for i in range(0, len(waits) - cap, cap):
                        _wf_counter[0] += 1
                        out.append(mybir.InstNoOp(
                            name=f"waitfix_{_wf_counter[0]}",
                            sync_info=mybir.SyncInfo(
                                on_wait=waits[i:i + cap], on_update=[]),
                            engine=inst.engine,
                            bass_nofuse=True,
                        ))
                    inst.sync_info = mybir.SyncInfo(
                        on_wait=keep,
                        on_update=list(si.on_update) if si else [])
                out.append(inst)
            if changed:
                bb.instructions = out
    return nc


# ---------------------------------------------------------------- program
def build_program(apply_waitfix=True):
    nc = bass.Bass()

    xqt = nc.dram_tensor("xqt", [D, S], F32R, kind="ExternalInput")
    xkt = nc.dram_tensor("xkt", [D, S], F32R, kind="ExternalInput")
    xvt = nc.dram_tensor("xvt", [D, S], F32R, kind="ExternalInput")
    wqt = nc.dram_tensor("wqt", [D, DH], F32R, kind="ExternalInput")
    wkt = nc.dram_tensor("wkt", [D, DH], F32R, kind="ExternalInput")
    wvt = nc.dram_tensor("wvt", [D, DH], F32R, kind="ExternalInput")
    wot = nc.dram_tensor("wot", [DH, D], F32R, kind="ExternalInput")
    bq2 = nc.dram_tensor("bq2", [P, 2], F32, kind="ExternalInput")
    bk2 = nc.dram_tensor("bk2", [P, 2], F32, kind="ExternalInput")
    bv2 = nc.dram_tensor("bv2", [P, 2], F32, kind="ExternalInput")
    bo_eff = nc.dram_tensor("bo_eff", [1, D], F32R, kind="ExternalInput")
    onesd = nc.dram_tensor("onesd", [1, P], F32R, kind="ExternalInput")
    onespv = nc.dram_tensor("onespv", [P, ST, GROUPS, 1], F32R,
                            kind="ExternalInput")
    y = nc.dram_tensor("y", [S, D], F32, kind="ExternalOutput")

    xqt_r = xqt.rearrange("(kt p) s -> kt p s", p=P)
    xkt_r = xkt.rearrange("(kt p) s -> kt p s", p=P)
    xvt_r = xvt.rearrange("(kt p) s -> kt p s", p=P)
    y_r = y.rearrange("(st p) d -> st p d", p=P)

    with tile.TileContext(nc) as tc:
      with ExitStack() as ctx:
        # ---- persistent SBUF ----
        wp = ctx.enter_context(tc.tile_pool(name="wp", bufs=1))
        wq_sb = wp.tile([P, KT, DH], F32R, tag="wq")
        wk_sb = wp.tile([P, KT, DH], F32R, tag="wk")
        wv_sb = wp.tile([P, KT, DH], F32R, tag="wv")
        wo_sb = wp.tile([P, 2, D], F32R, tag="wo")
        bq_sb = wp.tile([P, 2], F32, tag="bq")
        bk_sb = wp.tile([P, 2], F32, tag="bk")
        bv_sb = wp.tile([P, 2], F32, tag="bv")
        ones1 = wp.tile([1, P], F32R, tag="ones1")
        bo_sb = wp.tile([1, D], F32R, tag="bo")
        borep_sb = wp.tile([P, D], F32, tag="borep")

        qt_sb = wp.tile([P, 2, S], F32R, tag="qt")
        kt_sb = wp.tile([P, 2, S], F32R, tag="kt")
        pvw_sb = wp.tile([P, ST, GROUPS, DK + 1], F32R, tag="pvw")
        xn_sb = wp.tile([P, 2, S], F32R, tag="xn")

        nc.sync.dma_start(wq_sb[:], wqt.rearrange("(kt p) m -> p kt m", p=P))
        nc.sync.dma_start(wk_sb[:], wkt.rearrange("(kt p) m -> p kt m", p=P))
        nc.sync.dma_start(wv_sb[:], wvt.rearrange("(kt p) m -> p kt m", p=P))
        nc.sync.dma_start(wo_sb[:], wot.rearrange("(p2 p) d -> p p2 d", p=P))
        nc.sync.dma_start(bq_sb[:], bq2[:])
        nc.sync.dma_start(bk_sb[:], bk2[:])
        nc.sync.dma_start(bv_sb[:], bv2[:])
        nc.sync.dma_start(ones1[:], onesd[:])
        nc.sync.dma_start(bo_sb[:], bo_eff[:])
        nc.sync.dma_start(pvw_sb[:, :, :, DK:DK + 1], onespv[:])

        with nc.allow_low_precision(reason="float32r is fp32 rounded ~1e-4"):
          # ---------------- phase B: projections ----------------
          with (
              tc.tile_pool(name="xs", bufs=10) as xs,
              tc.tile_pool(name="pp", bufs=3, space="PSUM") as proj_ps,
              tc.tile_pool(name="vp", bufs=2, space="PSUM") as vproj_ps,
          ):
            # bo broadcast (rank-1) for the output stage
            for oc in range(2):
                bp = proj_ps.tile([P, CW], F32, tag="p")
                nc.tensor.matmul(bp[:], ones1[:], bo_sb[:, ds(CW * oc, CW)],
                                 start=True, stop=True)
                nc.vector.tensor_copy(borep_sb[:, ds(CW * oc, CW)], bp[:])

            for c in range(CH):
                csl = ds(CW * c, CW)
                qx, kx, vx = [], [], []
                for kt in range(KT):
                    t = xs.tile([P, CW], F32R, tag="xq")
                    nc.sync.dma_start(t[:], xqt_r[kt, :, csl])
                    qx.append(t)
                for kt in range(KT):
                    t = xs.tile([P, CW], F32R, tag="xk")
                    nc.sync.dma_start(t[:], xkt_r[kt, :, csl])
                    kx.append(t)
                for kt in range(KT):
                    t = xs.tile([P, CW], F32R, tag="xv")
                    nc.sync.dma_start(t[:], xvt_r[kt, :, csl])
                    vx.append(t)

                for p in range(2):
                    pp = proj_ps.tile([P, CW], F32, tag="p")
                    for kt in range(KT):
                        nc.tensor.matmul(pp[:], wq_sb[:, kt, ds(P * p, P)],
                                         qx[kt][:],
                                         start=(kt == 0), stop=(kt == KT - 1))
                    nc.vector.tensor_scalar_add(qt_sb[:, p, csl], pp[:],
                                                bq_sb[:, p:p + 1])
                for p in range(2):
                    pp = proj_ps.tile([P, CW], F32, tag="p")
                    for kt in range(KT):
                        nc.tensor.matmul(pp[:], wk_sb[:, kt, ds(P * p, P)],
                                         kx[kt][:],
                                         start=(kt == 0), stop=(kt == KT - 1))
                    nc.vector.tensor_scalar_add(kt_sb[:, p, csl], pp[:],
                                                bk_sb[:, p:p + 1])
                for st4 in range(4):
                    vp = vproj_ps.tile([P, DH], F32, tag="v")
                    for kt in range(KT):
                        nc.tensor.matmul(vp[:], vx[kt][:, ds(P * st4, P)],
                                         wv_sb[:, kt, :],
                                         start=(kt == 0), stop=(kt == KT - 1))
                    st = 4 * c + st4
                    nc.vector.tensor_copy(
                        pvw_sb[:, st, :, 0:DK],
                        vp[:].rearrange("p (h d) -> p h d", h=GROUPS))

          # ---------------- phase C: attention ----------------
          with (
              tc.tile_pool(name="ptp", bufs=4) as ptp,
              tc.tile_pool(name="rcps", bufs=2) as rcps,
              tc.tile_pool(name="reps", bufs=2) as repsb,
              tc.tile_pool(name="spp", bufs=2, space="PSUM") as sp_ps,
              tc.tile_pool(name="xap", bufs=2, space="PSUM") as xa_ps,
              tc.tile_pool(name="repp", bufs=2, space="PSUM") as rep_ps,
          ):
            for p in range(2):
                for c in range(CH):
                    csl = ds(CW * c, CW)
                    xaugs = [xa_ps.tile([P, CW], F32, tag="xaug",
                                        name=f"xaug_{p}_{c}_{i}")
                             for i in range(2)]
                    for grp in range(ST // 2):
                        for hh in range(2):
                            sp = sp_ps.tile([P, 2, CW], F32, tag="sp")
                            for j in range(2):
                                sk = 2 * grp + j
                                nc.tensor.matmul(
                                    sp[:, j, :],
                                    kt_sb[64 * hh:64 * hh + 64, p, ts(sk, P)],
                                    qt_sb[64 * hh:64 * hh + 64, p, csl],
                                    start=True, stop=True,
                                    tile_position=(64 * hh, 0))
                            pt = ptp.tile([P, 2, CW], F32R, tag="pt")
                            nc.scalar.activation(pt[:], sp[:], AF.Exp,
                                                 scale=0.125)
                            for j in range(2):
                                sk = 2 * grp + j
                                nc.tensor.matmul(
                                    xaugs[hh][0:DK + 1, :],
                                    pvw_sb[:, sk, 2 * p + hh, :],
                                    pt[:, j, :],
                                    start=(grp == 0 and j == 0),
                                    stop=(grp == ST // 2 - 1 and j == 1))
                    for hh in range(2):
                        rcp = rcps.tile([1, CW], F32R, tag="rcp")
                        nc.vector.reciprocal(rcp[:], xaugs[hh][DK:DK + 1, :])
                        rep = rep_ps.tile([P, CW], F32, tag="rep")
                        nc.tensor.matmul(rep[0:DK, :], ones1[:1, 0:DK],
                                         rcp[:], start=True, stop=True)
                        repc = repsb.tile([DK, CW], F32, tag="repc")
                        nc.vector.tensor_copy(repc[:], rep[0:DK, :])
                        xsl = xn_sb[64 * hh:64 * hh + 64, p, csl]
                        nc.vector.tensor_tensor(
                            xsl, xaugs[hh][0:DK, :], repc[:], ALU.mult)
                        nc.vector.tensor_scalar_add(
                            xsl, xsl,
                            bv_sb[64 * hh:64 * hh + 64, p:p + 1])

          # ---------------- phase D: output projection ----------------
          with (
              tc.tile_pool(name="ev", bufs=4) as ev,
              tc.tile_pool(name="yp", bufs=4, space="PSUM") as y_ps,
          ):
            for st in range(ST):
                for oc in range(2):
                    yp = y_ps.tile([P, CW], F32, tag="y")
                    for p2 in range(2):
                        nc.tensor.matmul(yp[:], xn_sb[:, p2, ts(st, P)],
                                         wo_sb[:, p2, ds(CW * oc, CW)],
                                         start=(p2 == 0), stop=(p2 == 1))
                    ysb = ev.tile([P, CW], F32, tag="ysb")
                    nc.vector.tensor_tensor(ysb[:], yp[:],
                                            borep_sb[:, ds(CW * oc, CW)],
                                            ALU.add)
                    nc.sync.dma_start(y_r[st, :, ds(CW * oc, CW)], ysb[:])

    if apply_waitfix:
        _split_waits(nc, cap=1)
    return nc


_program_cache = {}


def get_program():
    if "nc" not in _program_cache:
        _program_cache["nc"] = build_program()
    return _program_cache["nc"]


def make_in_maps(Q, K, V, Wq, bq, Wk, bk, Wv, bv, Wo, bo):
    Q = np.asarray(Q, dtype=np.float32)
    K = np.asarray(K, dtype=np.float32)
    V = np.asarray(V, dtype=np.float32)
    Wq = np.asarray(Wq, dtype=np.float32)
    Wk = np.asarray(Wk, dtype=np.float32)
    Wv = np.asarray(Wv, dtype=np.float32)
    Wo = np.asarray(Wo, dtype=np.float32)
    bq = np.asarray(bq, dtype=np.float32)
    bk = np.asarray(bk, dtype=np.float32)
    bv = np.asarray(bv, dtype=np.float32)
    bo = np.asarray(bo, dtype=np.float32)

    xt = {b: {
        "q": np.ascontiguousarray(Q[b].T),
        "k": np.ascontiguousarray(K[b].T),
        "v": np.ascontiguousarray(V[b].T),
    } for b in range(B)}

    ones1 = np.ones((1, P), dtype=np.float32)
    onespv = np.ones((P, ST, GROUPS, 1), dtype=np.float32)
    zero_bo = np.zeros((1, D), dtype=np.float32)
    bo_row = bo.reshape(1, D)

    in_maps = []
    for c in range(NCORES):
        b, g = divmod(c, GROUPS)
        hs = slice(DH * g, DH * (g + 1))
        in_maps.append({
            "xqt": xt[b]["q"],
            "xkt": xt[b]["k"],
            "xvt": xt[b]["v"],
            "wqt": np.ascontiguousarray(Wq[hs, :].T),
            "wkt": np.ascontiguousarray(Wk[hs, :].T),
            "wvt": np.ascontiguousarray(Wv[hs, :].T),
            "wot": np.ascontiguousarray(Wo[:, hs].T),
            "bq2": np.ascontiguousarray(bq[hs].reshape(2, P).T),
            "bk2": np.ascontiguousarray(bk[hs].reshape(2, P).T),
            "bv2": np.ascontiguousarray(bv[hs].reshape(2, P).T),
            "bo_eff": bo_row if g == 0 else zero_bo,
            "onesd": ones1,
            "onespv": onespv,
        })
    return in_maps


def combine_outputs(results):
    """results: list of 8 per-core dicts with 'y' [S, D] partials."""
    out = np.zeros((B, S, D), dtype=np.float32)
    for c, res in enumerate(results):
        b = c // GROUPS
        out[b] += res["y"]
    return out


def kernel(**inputs) -> np.ndarray:
    nc = get_program()
    in_maps = make_in_maps(**inputs)
    res = run_bass_kernel_spmd(nc, in_maps, core_ids=list(range(NCORES)))
    return combine_outputs(res.results)



# revision 7
# speedup vs baseline: 1.4303x; 1.4303x over previous
"""Multi-head attention (nn_GroupQueryAttention_163208757512) on 8 TRN2 cores.

Problem: B=2, S=2048, D=1024, H=16 heads, DK=64. f32 in/out.
    q = Q @ Wq.T + bq  (per head)   k, v likewise
    out = softmax(q k^T / 8) v  -> concat heads -> @ Wo.T + bo

Sharding: core c handles batch b=c//4 and head group g=c%4 (4 heads,
feature slice hs = 256*g : 256*g+256). Data parallel on B, tensor
parallel on heads; per-core partial outputs are host-summed.

v2 design (ACT-bound: exp on ScalarE is the ~147us floor; everything
else hides under it):
  - all matmul operands bf16 (host casts); FWL halves weight loads.
  - full X^T tensors staged in SBUF via 3 large DMAs.
  - scores for a head PAIR packed into one [128, 2x512] PSUM tile
    (rows 0-63 head A via tile_position (0,0), rows 64-127 head B via
    (64,0) - the two MMs run concurrently); ONE exp instr per sk tile
    covers both heads (N=1024 amortizes ACT's 352-cycle overhead).
  - sk loop software-pipelined: scores(sk+1) emitted BEFORE pv(sk) so
    the PE FIFO never head-of-line-blocks behind an exp.
  - q-proj(qc+1) and out-proj(qc-1) matmuls interleaved one-per-sk as
    PE gap fillers inside the attention loop.
  - PV uses the stationary [v_h | 1] trick (M=65): column 64
    accumulates softmax denominators for free.
  - 1/denom via reciprocal_approx_fast (18-bit, 5x faster), broadcast
    across partitions with a K=1 ones matmul, applied on DVE.
  - bv folded into bo_eff on host (bv @ Wo_slice.T); bo enters via a
    rank-1 ones x bo_eff product added during y evacuation.
  - PSUM budget exactly 8 banks: scores 2x2, xaug 2, q-proj 1,
    y/rep/bo shared 1.

Toolchain constraints: walrus allows ONE sync-wait per instruction
(split_waits post-pass chains the rest onto NoOps); accumulation
groups must keep one lhsT base partition.
"""

import numpy as np
from contextlib import ExitStack

import concourse.bass as bass
import concourse.mybir as mybir
import concourse.tile as tile
from concourse.bass import ds, ts
from concourse.bass_utils import run_bass_kernel_spmd

F32 = mybir.dt.float32
F32R = mybir.dt.float32r
BF16 = mybir.dt.bfloat16
AF = mybir.ActivationFunctionType
ALU = mybir.AluOpType

B, S, D, H = 2, 2048, 1024, 16
DK = D // H            # 64
NCORES = 8
GROUPS = 4             # head groups per batch
DH = D // GROUPS       # 256 feature cols per core
P = 128
KT = D // P            # 8 contraction tiles for projections
ST = S // P            # 16 s-tiles
CH = 4                 # q chunks
CW = S // CH           # 512


# ---------------------------------------------------------------- wait fix
_wf_counter = [0]


def _split_waits(nc, cap=1):
    """walrus in this container accepts at most one sync-wait command per
    instruction; chain the rest onto same-engine NoOps placed just before."""
    for fn in nc.m.functions:
        for bb in fn.blocks:
            out, changed = [], False
            for inst in bb.instructions:
                si = inst.sync_info
                waits = list(si.on_wait) if (si is not None and si.on_wait) else []
                if len(waits) > cap:
                    changed = True
                    keep = waits[-cap:]
                    for i in range(0, len(waits) - cap, cap):
                        _wf_counter[0] += 1
                        out.append(mybir.InstNoOp(
                            name=f"waitfix_{_wf_counter[0]}",
                            sync_info=mybir.SyncInfo(
                                on_wait=waits[i:i + cap], on_update=[]),
                            engine=inst.engine,
                            bass_nofuse=True,
                        ))
                    inst.sync_info = mybir.SyncInfo(
                        on_wait=keep,
                        on_update=list(si.on_update) if si else [])
                out.append(inst)
            if changed:
                bb.instructions = out
    return nc


# ---------------------------------------------------------------- program
def build_program(apply_waitfix=True):
    nc = bass.Bass()

    xqt = nc.dram_tensor("xqt", [D, S], BF16, kind="ExternalInput")
    xkt = nc.dram_tensor("xkt", [D, S], BF16, kind="ExternalInput")
    xvt = nc.dram_tensor("xvt", [D, S], BF16, kind="ExternalInput")
    wqt = nc.dram_tensor("wqt", [D, DH], BF16, kind="ExternalInput")
    wkt = nc.dram_tensor("wkt", [D, DH], BF16, kind="ExternalInput")
    wvt = nc.dram_tensor("wvt", [D, DH], BF16, kind="ExternalInput")
    wot = nc.dram_tensor("wot", [DH, D], BF16, kind="ExternalInput")
    bq2 = nc.dram_tensor("bq2", [P, 2], F32, kind="ExternalInput")
    bk2 = nc.dram_tensor("bk2", [P, 2], F32, kind="ExternalInput")
    bo_eff = nc.dram_tensor("bo_eff", [1, D], F32R, kind="ExternalInput")
    onesd = nc.dram_tensor("onesd", [1, P], F32R, kind="ExternalInput")
    onespv = nc.dram_tensor("onespv", [P, ST, GROUPS, 1], BF16,
                            kind="ExternalInput")
    y = nc.dram_tensor("y", [S, D], F32, kind="ExternalOutput")

    xq_r = xqt.rearrange("(kt p) s -> p kt s", p=P)
    xk_r = xkt.rearrange("(kt p) s -> p kt s", p=P)
    xv_r = xvt.rearrange("(kt p) s -> p kt s", p=P)
    y_r = y.rearrange("(st p) d -> st p d", p=P)

    with tile.TileContext(nc) as tc:
      with ExitStack() as ctx:
        # ---- persistent SBUF ----
        wp = ctx.enter_context(tc.tile_pool(name="wp", bufs=1))
        wq_sb = wp.tile([P, KT, DH], BF16, tag="wq")
        wk_sb = wp.tile([P, KT, DH], BF16, tag="wk")
        wv_sb = wp.tile([P, KT, DH], BF16, tag="wv")
        wo_sb = wp.tile([P, 2, D], BF16, tag="wo")
        bq_sb = wp.tile([P, 2], F32, tag="bq")
        bk_sb = wp.tile([P, 2], F32, tag="bk")
        ones1 = wp.tile([1, P], F32R, tag="ones1")
        bo_sb = wp.tile([1, D], F32R, tag="bo")
        borep_sb = wp.tile([P, D], F32, tag="borep")

        xq_sb = wp.tile([P, KT, S], BF16, tag="xq")
        xk_sb = wp.tile([P, KT, S], BF16, tag="xk")
        xv_sb = wp.tile([P, KT, S], BF16, tag="xv")

        qt_sb = wp.tile([P, 2, S], BF16, tag="qt")
        kt_sb = wp.tile([P, 2, S], BF16, tag="kt")
        pvw_sb = wp.tile([P, ST, GROUPS, DK + 1], BF16, tag="pvw")
        xn_sb = wp.tile([P, 2, S], BF16, tag="xn")

        nc.sync.dma_start(wk_sb[:], wkt.rearrange("(kt p) m -> p kt m", p=P))
        nc.sync.dma_start(wv_sb[:], wvt.rearrange("(kt p) m -> p kt m", p=P))
        nc.sync.dma_start(wq_sb[:], wqt.rearrange("(kt p) m -> p kt m", p=P))
        nc.sync.dma_start(wo_sb[:], wot.rearrange("(p2 p) d -> p p2 d", p=P))
        nc.sync.dma_start(bq_sb[:], bq2[:])
        nc.sync.dma_start(bk_sb[:], bk2[:])
        nc.sync.dma_start(ones1[:], onesd[:])
        nc.sync.dma_start(bo_sb[:], bo_eff[:])
        nc.sync.dma_start(pvw_sb[:, :, :, DK:DK + 1], onespv[:])
        nc.sync.dma_start(xk_sb[:], xk_r)
        nc.sync.dma_start(xv_sb[:], xv_r)
        nc.sync.dma_start(xq_sb[:], xq_r)

        with nc.allow_low_precision(reason="bf16 kernel, rel tol 2e-2"):
          with (
              tc.tile_pool(name="qp", bufs=1, space="PSUM") as q_ps,     # 1 bank
              tc.tile_pool(name="yp", bufs=1, space="PSUM") as y_ps,     # 1 bank
          ):
            with (
                tc.tile_pool(name="kp", bufs=1, space="PSUM") as k_ps,   # 4 banks
                tc.tile_pool(name="vp", bufs=2, space="PSUM") as v_ps,   # 2x1 banks
            ):
              # ---------------- prologue: k-proj (weight-stationary) ----
              for p2 in range(2):
                kps = k_ps.tile([P, S], F32, tag="k")
                for kt in range(KT):
                    for c4 in range(CH):
                        nc.tensor.matmul(kps[:, ds(CW * c4, CW)],
                                         wk_sb[:, kt, ds(P * p2, P)],
                                         xk_sb[:, kt, ds(CW * c4, CW)],
                                         start=(kt == 0), stop=(kt == KT - 1))
                for c4 in range(CH):
                    nc.vector.tensor_scalar_add(
                        kt_sb[:, p2, ds(CW * c4, CW)],
                        kps[:, ds(CW * c4, CW)], bk_sb[:, p2:p2 + 1])

              # ---------------- prologue: v-proj (x-stationary) ---------
              for st in range(ST):
                vp = v_ps.tile([P, DH], F32, tag="v")
                for kt in range(KT):
                    nc.tensor.matmul(vp[:], xv_sb[:, kt, ts(st, P)],
                                     wv_sb[:, kt, :],
                                     start=(kt == 0), stop=(kt == KT - 1))
                nc.vector.tensor_copy(
                    pvw_sb[:, st, :, 0:DK],
                    vp[:].rearrange("p (h d) -> p h d", h=GROUPS))

            # ---------------- prologue: bo broadcast, q-proj qc0 --------
            for oc in range(2):
                bp = y_ps.tile([P, CW], F32, tag="y")
                nc.tensor.matmul(bp[:], ones1[:], bo_sb[:, ds(CW * oc, CW)],
                                 start=True, stop=True)
                nc.vector.tensor_copy(borep_sb[:, ds(CW * oc, CW)], bp[:])

            def qproj_thunks(qc):
                """16 matmuls + 2 evacs for q-projection of chunk qc, as
                one-matmul closures for PE gap-filling."""
                thunks = []
                state = {}
                for p2 in range(2):
                    for kt in range(KT):
                        def mk(p2=p2, kt=kt):
                            if kt == 0:
                                state[p2] = q_ps.tile([P, CW], F32, tag="q",
                                                      name=f"qps_{qc}_{p2}")
                            nc.tensor.matmul(state[p2][:],
                                             wq_sb[:, kt, ds(P * p2, P)],
                                             xq_sb[:, kt, ds(CW * qc, CW)],
                                             start=(kt == 0),
                                             stop=(kt == KT - 1))
                            if kt == KT - 1:
                                nc.vector.tensor_scalar_add(
                                    qt_sb[:, p2, ds(CW * qc, CW)],
                                    state[p2][:], bq_sb[:, p2:p2 + 1])
                        thunks.append(mk)
                return thunks

            def yproj_thunks(qc):
                """Out-projection for the 4 s-tiles of chunk qc: each thunk
                is one (st, oc) tile = 2 accum matmuls + evac + store."""
                thunks = []
                for st4 in range(4):
                    st = 4 * qc + st4
                    for oc in range(2):
                        def mk(st=st, oc=oc):
                            yp = y_ps.tile([P, CW], F32, tag="y",
                                           name=f"yps_{st}_{oc}")
                            for p2 in range(2):
                                nc.tensor.matmul(
                                    yp[:], xn_sb[:, p2, ts(st, P)],
                                    wo_sb[:, p2, ds(CW * oc, CW)],
                                    start=(p2 == 0), stop=(p2 == 1))
                            ysb = ev_pool.tile([P, CW], F32, tag="ysb",
                                               name=f"ysb_{st}_{oc}")
                            nc.vector.tensor_tensor(
                                ysb[:], yp[:], borep_sb[:, ds(CW * oc, CW)],
                                ALU.add)
                            nc.sync.dma_start(y_r[st, :, ds(CW * oc, CW)],
                                              ysb[:])
                        thunks.append(mk)
                return thunks

            # ---------------- steady state: attention -------------------
            with (
                tc.tile_pool(name="ptp", bufs=3) as ptp,
                tc.tile_pool(name="rcps", bufs=2) as rcps,
                tc.tile_pool(name="reps", bufs=2) as repsb,
                tc.tile_pool(name="evp", bufs=3) as ev_pool,
                tc.tile_pool(name="spp", bufs=2, space="PSUM") as sp_ps,   # 2x2 banks
                tc.tile_pool(name="xap", bufs=2, space="PSUM") as xa_ps,   # 2x1 banks
            ):
                filler = []          # deque of one-matmul closures
                for t in qproj_thunks(0):
                    t()              # q-proj qc0 now (PE otherwise idle)

                def score_mm(pair, qc, sk, sp):
                    for hh in range(2):
                        nc.tensor.matmul(
                            sp[:, hh, :],
                            kt_sb[64 * hh:64 * hh + 64, pair, ts(sk, P)],
                            qt_sb[64 * hh:64 * hh + 64, pair,
                                  ds(CW * qc, CW)],
                            start=True, stop=True,
                            tile_position=(64 * hh, 0))

                def norm_thunks(pair, qc, xaugs):
                    """Normalization closures for one section; deferred past
                    the next section's first score MMs to avoid stalling ACT
                    behind the PE FIFO."""
                    thunks = []
                    for hh in range(2):
                        def mk(hh=hh):
                            rcp = rcps.tile([1, CW], F32R, tag="rcp",
                                            name=f"rcp_{pair}_{qc}_{hh}")
                            nc.vector.reciprocal(rcp[:],
                                                 xaugs[hh][DK:DK + 1, :])
                            rep = y_ps.tile([P, CW], F32, tag="y",
                                            name=f"rep_{pair}_{qc}_{hh}")
                            nc.tensor.matmul(rep[0:DK, :], ones1[:1, 0:DK],
                                             rcp[:], start=True, stop=True)
                            repc = repsb.tile([DK, CW], F32, tag="repc",
                                              name=f"repc_{pair}_{qc}_{hh}")
                            nc.vector.tensor_copy(repc[:], rep[0:DK, :])
                            nc.vector.tensor_tensor(
                                xn_sb[64 * hh:64 * hh + 64, pair,
                                      ds(CW * qc, CW)],
                                xaugs[hh][0:DK, :], repc[:], ALU.mult)
                        thunks.append(mk)
                    return thunks

                pending_norm = []
                for qc in range(CH):
                    for pair in range(2):
                        # refill the filler queue at section boundaries
                        if pair == 0 and qc + 1 < CH:
                            filler.extend(qproj_thunks(qc + 1))
                        if pair == 1 and qc >= 1:
                            filler.extend(yproj_thunks(qc - 1))

                        sps = []
                        sp0 = sp_ps.tile([P, 2, CW], F32, tag="sp")
                        score_mm(pair, qc, 0, sp0)
                        sps.append(sp0)
                        # previous section's normalization: emitted after
                        # this section's first scores so ACT never waits
                        for t in pending_norm:
                            t()
                        pending_norm = []
                        xaugs = [xa_ps.tile([DK + 1, CW], F32, tag="xa",
                                            name=f"xa_{qc}_{pair}_{i}")
                                 for i in range(2)]
                        for sk in range(ST):
                            if sk + 1 < ST:
                                spn = sp_ps.tile([P, 2, CW], F32, tag="sp")
                                score_mm(pair, qc, sk + 1, spn)
                                sps.append(spn)
                            sp = sps[sk]
                            pt = ptp.tile([P, 2, CW], BF16, tag="pt")
                            nc.scalar.activation(pt[:], sp[:], AF.Exp,
                                                 scale=0.125)
                            if filler:
                                filler.pop(0)()
                            for hh in range(2):
                                nc.tensor.matmul(
                                    xaugs[hh][:],
                                    pvw_sb[:, sk, 2 * pair + hh, :],
                                    pt[:, hh, :],
                                    start=(sk == 0), stop=(sk == ST - 1))
                        pending_norm = norm_thunks(pair, qc, xaugs)

                # drain: last section's norm, leftover fillers, final y
                for t in pending_norm:
                    t()
                for t in filler:
                    t()
                for t in yproj_thunks(CH - 1):
                    t()

    if apply_waitfix:
        _split_waits(nc, cap=1)
    return nc


_program_cache = {}


def get_program():
    if "nc" not in _program_cache:
        _program_cache["nc"] = build_program()
    return _program_cache["nc"]


def make_in_maps(Q, K, V, Wq, bq, Wk, bk, Wv, bv, Wo, bo):
    import ml_dtypes
    bf16 = ml_dtypes.bfloat16

    Q = np.asarray(Q, dtype=np.float32)
    K = np.asarray(K, dtype=np.float32)
    V = np.asarray(V, dtype=np.float32)
    Wq = np.asarray(Wq, dtype=np.float32)
    Wk = np.asarray(Wk, dtype=np.float32)
    Wv = np.asarray(Wv, dtype=np.float32)
    Wo = np.asarray(Wo, dtype=np.float32)
    bq = np.asarray(bq, dtype=np.float32)
    bk = np.asarray(bk, dtype=np.float32)
    bv = np.asarray(bv, dtype=np.float32)
    bo = np.asarray(bo, dtype=np.float32)

    xt = {b: {
        "q": np.ascontiguousarray(Q[b].T).astype(bf16),
        "k": np.ascontiguousarray(K[b].T).astype(bf16),
        "v": np.ascontiguousarray(V[b].T).astype(bf16),
    } for b in range(B)}

    ones1 = np.ones((1, P), dtype=np.float32)
    onespv = np.ones((P, ST, GROUPS, 1), dtype=bf16)

    in_maps = []
    for c in range(NCORES):
        b, g = divmod(c, GROUPS)
        hs = slice(DH * g, DH * (g + 1))
        # fold bv's contribution to y (bv @ Wo_slice.T) into bo_eff;
        # bo itself only on the g==0 core of each batch.
        bo_c = (bo if g == 0 else 0.0) + Wo[:, hs] @ bv[hs]
        in_maps.append({
            "xqt": xt[b]["q"],
            "xkt": xt[b]["k"],
            "xvt": xt[b]["v"],
            "wqt": np.ascontiguousarray(Wq[hs, :].T).astype(bf16),
            "wkt": np.ascontiguousarray(Wk[hs, :].T).astype(bf16),
            "wvt": np.ascontiguousarray(Wv[hs, :].T).astype(bf16),
            "wot": np.ascontiguousarray(Wo[:, hs].T).astype(bf16),
            "bq2": np.ascontiguousarray(bq[hs].reshape(2, P).T),
            "bk2": np.ascontiguousarray(bk[hs].reshape(2, P).T),
            "bo_eff": np.ascontiguousarray(bo_c.reshape(1, D)),
            "onesd": ones1,
            "onespv": onespv,
        })
    return in_maps


def combine_outputs(results):
    """results: list of 8 per-core dicts with 'y' [S, D] partials."""
    out = np.zeros((B, S, D), dtype=np.float32)
    for c, res in enumerate(results):
        b = c // GROUPS
        out[b] += res["y"]
    return out


def kernel(**inputs) -> np.ndarray:
    nc = get_program()
    in_maps = make_in_maps(**inputs)
    res = run_bass_kernel_spmd(nc, in_maps, core_ids=list(range(NCORES)))
    return combine_outputs(res.results)
